# revision 1
# baseline (speedup 1.0000x reference)
"""MultiHeadLatentAttention on 8 trn2 NeuronCores (Bass/Tile).

Sharding: core = (b, qc) with b = core//4 (data parallel over batch),
qc = core%4 (query-chunk of 512 positions). Each core computes the full
K/V for its batch (replicated within the 4-core batch group) and the
attention + output projection for its 512 queries; the host gather is a
pure concatenation.

Device layouts (partition dim first):
  hT      [H, S]   hidden[b]^T fp16 (host prep)
  c_kvT   [LAT, S] latent KV, fp16
  k_cT_h  [D, Sk]  per head fp16 (per key-block)
  v_c     [Sk, DH] natural fp16 (per key-block)
  scoresT [Sk, SQ] keys on partitions; softmax normalized late via
                   column sums (ones-matmul) + 1/sum broadcast
  ctxT    [DH, SQ] f32 accumulator -> fp16 normalized -> W_O (fp16)
Projections contract over H/LAT in PSUM fp32. exp() runs on ACT with the
key mask + a constant logit shift folded into its bias.
"""
import math
import numpy as np
from contextlib import ExitStack

import concourse.bass as bass
import concourse.mybir as mybir
import concourse.tile as tile
from concourse.bass_utils import run_bass_kernel_spmd

F32R = mybir.dt.float32r
F32 = mybir.dt.float32
F16 = mybir.dt.float16
AF = mybir.ActivationFunctionType

H, NH, LAT = 2048, 16, 512
D = H // NH            # 128
B, S = 2, 2048
SQ = S // 4            # 512 queries per core
E = H // 128           # 16 contraction chunks over H
L = LAT // 128         # 4 chunks over LAT
KBLK = 512             # key block
NKB = S // KBLK        # 4
KSUB = KBLK // 128     # 4 key sub-chunks of 128
SCALE = 1.0 / math.sqrt(D)
SHIFT = 8.0            # softmax logit shift (cancels in normalization)

_n_split = 0


def _split_multi_waits(nc):
    """walrus in this container allows ONE sync wait per instruction; Tile
    attaches several. Hoist extras onto single-wait NoOps just before."""
    global _n_split
    total = 0
    for f in nc.m.functions:
        for bb in f.blocks:
            out = []
            changed = False
            for inst in bb.instructions:
                si = inst.sync_info
                if si is not None and len(si.on_wait) > 1:
                    changed = True
                    waits = list(si.on_wait)
                    for w in waits[:-1]:
                        _n_split += 1
                        total += 1
                        nop = mybir.InstNoOp(
                            name=f"wsplit-{_n_split}", text_hint="waitsplit")
                        nop.engine = inst.engine
                        nop.sync_info = mybir.SyncInfo(on_wait=[w], on_update=[])
                        nc.register_instruction(nop)
                        out.append(nop)
                    inst.sync_info = mybir.SyncInfo(
                        on_wait=[waits[-1]], on_update=list(si.on_update))
                out.append(inst)
            if changed:
                bb.instructions = out
    return total


def build(reps=1):
    nc = bass.Bass()

    d_hT = nc.dram_tensor("hT", [H, S], F16, kind="ExternalInput")
    d_hTq = nc.dram_tensor("hTq", [H, SQ], F16, kind="ExternalInput")
    d_wdkv = nc.dram_tensor("wdkv", [H, LAT], F16, kind="ExternalInput")
    d_wdq = nc.dram_tensor("wdq", [H, LAT], F16, kind="ExternalInput")
    d_wkr = nc.dram_tensor("wkr", [H, D], F16, kind="ExternalInput")
    d_wqr = nc.dram_tensor("wqr", [H, D], F16, kind="ExternalInput")
    d_wuk = nc.dram_tensor("wuk", [LAT, H], F16, kind="ExternalInput")
    d_wuv = nc.dram_tensor("wuv", [LAT, H], F16, kind="ExternalInput")
    d_wuq = nc.dram_tensor("wuq", [LAT, H], F16, kind="ExternalInput")
    d_wo = nc.dram_tensor("wo", [H, H], F16, kind="ExternalInput")
    d_cos2 = nc.dram_tensor("cos2", [D, S], F16, kind="ExternalInput")
    d_sins = nc.dram_tensor("sins", [D, S], F16, kind="ExternalInput")
    d_cos2q = nc.dram_tensor("cos2q", [D, SQ], F16, kind="ExternalInput")
    d_sinsq = nc.dram_tensor("sinsq", [D, SQ], F16, kind="ExternalInput")
    d_maskb = nc.dram_tensor("maskb", [128, E], F32, kind="ExternalInput")
    d_bdkv = nc.dram_tensor("bdkv", [128, L], F32, kind="ExternalInput")
    d_bdq = nc.dram_tensor("bdq", [128, L], F32, kind="ExternalInput")
    d_buk = nc.dram_tensor("buk", [128, NH], F32, kind="ExternalInput")
    d_buq = nc.dram_tensor("buq", [128, NH], F32, kind="ExternalInput")
    d_buv = nc.dram_tensor("buv", [128, NH], F32, kind="ExternalInput")
    d_bkr = nc.dram_tensor("bkr", [128, 1], F32, kind="ExternalInput")
    d_bqr = nc.dram_tensor("bqr", [128, 1], F32, kind="ExternalInput")
    d_bo = nc.dram_tensor("bo", [1, H], F32R, kind="ExternalInput")
    d_ones = nc.dram_tensor("ones128", [128, 1], F16, kind="ExternalInput")
    d_onesr = nc.dram_tensor("onesrow", [1, 128], F32R, kind="ExternalInput")
    d_out = nc.dram_tensor("out", [SQ, H], F32, kind="ExternalOutput")

    with tile.TileContext(nc) as tc, ExitStack() as es:
        perm = es.enter_context(tc.tile_pool(name="perm", bufs=1))

        ckvT = perm.tile([128, L, S], F16)         # 16 KB/p
        qcT = perm.tile([128, NH, SQ], F16, tag="qslot")   # 16
        ctxa = perm.tile([128, NH, SQ], F32)       # 32
        # ctxf shares qcT's slot (WAR-safe: all scores reads precede the
        # normalize writes). With reps>1 (timing builds) that sharing is
        # circular across reps, so fall back to a separate tile + single
        # K/V buffering there.
        ctxf = perm.tile([128, NH, SQ], F16,
                         tag="qslot" if reps == 1 else "ctxf")
        sums = perm.tile([1, NH, SQ], F16)         # 16
        krro = perm.tile([128, S], F16)            # 4
        qrro = perm.tile([128, SQ], F16)           # 1

        maskb = perm.tile([128, E], F32)
        bdkv = perm.tile([128, L], F32)
        bdq = perm.tile([128, L], F32)
        buk = perm.tile([128, NH], F32)
        buq = perm.tile([128, NH], F32)
        buv = perm.tile([128, NH], F32)
        bkr = perm.tile([128, 1], F32)
        bqr = perm.tile([128, 1], F32)
        bo = perm.tile([1, H], F32R)
        ones = perm.tile([128, 1], F16)
        onesr = perm.tile([1, 128], F32R)
        consts = [(maskb, d_maskb), (bdkv, d_bdkv), (bdq, d_bdq),
                  (buk, d_buk), (buq, d_buq), (buv, d_buv),
                  (bkr, d_bkr), (bqr, d_bqr), (bo, d_bo),
                  (ones, d_ones), (onesr, d_onesr)]

        for _rep in range(reps):
            # ============ Phase A: all projections, one hT pass ============
            with tc.tile_pool(name="stA", bufs=1) as stA, \
                 tc.tile_pool(name="psA", bufs=1, space="PSUM") as psA:
                wdkv = stA.tile([128, E, LAT], F16, tag="wdkv_slot")
                wkr = stA.tile([128, E, D], F16)
                wdq = stA.tile([128, E, LAT], F16)
                wqr = stA.tile([128, E, D], F16)
                wuq = stA.tile([128, L, H], F16,
                               tag="wuq" if reps == 1 else "wdkv_slot")
                cqT = stA.tile([128, L, SQ], F16)
                krraw = stA.tile([128, S], F32, tag="krraw")

                # --- A1: c_kvT + raw k_rT, single pass over hT s-quarters ---
                for sq in range(4):
                    ps = [psA.tile([128, 512], F32, tag=f"ckv{l}", name=f"ckv{l}")
                          for l in range(L)]
                    kps = psA.tile([128, 512], F32, tag="krp", name="krp")
                    for e in range(E):
                        if sq == 0:
                            nc.sync.dma_start(
                                out=wdkv[:, e, :],
                                in_=d_wdkv[e * 128:(e + 1) * 128, :])
                            nc.sync.dma_start(
                                out=wkr[:, e, :],
                                in_=d_wkr[e * 128:(e + 1) * 128, :])
                        ht = stA.tile([128, 512], F16, tag="htA1", bufs=6)
                        eng = nc.sync if e % 2 == 0 else nc.gpsimd
                        eng.dma_start(
                            out=ht[:],
                            in_=d_hT[e * 128:(e + 1) * 128,
                                     sq * 512:(sq + 1) * 512])
                        for l in range(L):
                            nc.tensor.matmul(
                                ps[l][:], wdkv[:, e, l * 128:(l + 1) * 128],
                                ht[:], start=(e == 0), stop=(e == E - 1))
                        nc.tensor.matmul(kps[:], wkr[:, e, :], ht[:],
                                         start=(e == 0), stop=(e == E - 1))
                    if sq == 0:
                        if _rep == 0:
                            for t, d in consts:
                                nc.gpsimd.dma_start(out=t[:], in_=d[:])
                        # prefetch q-side operands behind the first quarter
                        nc.gpsimd.dma_start(
                            out=wdq[:],
                            in_=d_wdq.rearrange("(e p) l -> p e l", p=128))
                        if reps == 1:
                            nc.gpsimd.dma_start(
                                out=wuq[:],
                                in_=d_wuq.rearrange("(l p) h -> p l h", p=128))
                        nc.gpsimd.dma_start(
                            out=wqr[:],
                            in_=d_wqr.rearrange("(e p) d -> p e d", p=128))
                    for l in range(L):
                        nc.scalar.activation(
                            out=ckvT[:, l, sq * 512:(sq + 1) * 512],
                            in_=ps[l][:], func=AF.Identity,
                            bias=bdkv[:, l:l + 1], scale=1.0)
                    nc.scalar.activation(
                        out=krraw[:, sq * 512:(sq + 1) * 512], in_=kps[:],
                        func=AF.Identity, bias=bkr[:], scale=1.0)

                # --- A2: rope(k_r): out = raw*cos2 + swap(raw)*sins ---
                cos2 = stA.tile([128, S], F16, tag="cos2")
                sins = stA.tile([128, S], F16, tag="sins")
                nc.sync.dma_start(out=cos2[:], in_=d_cos2[:])
                nc.sync.dma_start(out=sins[:], in_=d_sins[:])
                krsw = stA.tile([128, S], F32, tag="krsw")
                nc.gpsimd.dma_start(out=krsw[0:64, :], in_=krraw[64:128, :])
                nc.gpsimd.dma_start(out=krsw[64:128, :], in_=krraw[0:64, :])
                nc.vector.tensor_mul(krraw[:], krraw[:], cos2[:])
                nc.vector.tensor_mul(krsw[:], krsw[:], sins[:])
                nc.vector.tensor_add(krro[:], krraw[:], krsw[:])

                # --- A3: c_qT + raw q_rT from resident hTq ---
                htq = stA.tile([128, E, SQ], F16, tag="htq", name="htq")
                nc.gpsimd.dma_start(
                    out=htq[:], in_=d_hTq.rearrange("(e p) q -> p e q", p=128))
                qps = psA.tile([128, SQ], F32, tag="qrx", name="qrp")
                for e in range(E):
                    nc.tensor.matmul(qps[:], wqr[:, e, :], htq[:, e, :],
                                     start=(e == 0), stop=(e == E - 1))
                qrraw = stA.tile([128, SQ], F32, tag="qrraw")
                nc.scalar.activation(out=qrraw[:], in_=qps[:],
                                     func=AF.Identity, bias=bqr[:], scale=1.0)
                for lg in range(2):
                    cps = [psA.tile([128, SQ], F32, tag=f"cq{j}", name=f"cqp{j}")
                           for j in range(2)]
                    for e in range(E):
                        for j in range(2):
                            l = lg * 2 + j
                            nc.tensor.matmul(
                                cps[j][:], wdq[:, e, l * 128:(l + 1) * 128],
                                htq[:, e, :], start=(e == 0), stop=(e == E - 1))
                    for j in range(2):
                        l = lg * 2 + j
                        nc.scalar.activation(out=cqT[:, l, :], in_=cps[j][:],
                                             func=AF.Identity,
                                             bias=bdq[:, l:l + 1], scale=1.0)
                cos2q = stA.tile([128, SQ], F16, tag="cos2q")
                sinsq = stA.tile([128, SQ], F16, tag="sinsq")
                nc.sync.dma_start(out=cos2q[:], in_=d_cos2q[:])
                nc.sync.dma_start(out=sinsq[:], in_=d_sinsq[:])
                qrsw = stA.tile([128, SQ], F32, tag="qrsw")
                nc.gpsimd.dma_start(out=qrsw[0:64, :], in_=qrraw[64:128, :])
                nc.gpsimd.dma_start(out=qrsw[64:128, :], in_=qrraw[0:64, :])
                nc.vector.tensor_mul(qrraw[:], qrraw[:], cos2q[:])
                nc.vector.tensor_mul(qrsw[:], qrsw[:], sinsq[:])
                nc.vector.tensor_add(qrro[:], qrraw[:], qrsw[:])

                # --- A5: q_cT (fp16) = W_UQ^T @ c_qT ---
                if reps != 1:
                    nc.gpsimd.dma_start(
                        out=wuq[:],
                        in_=d_wuq.rearrange("(l p) h -> p l h", p=128))
                for h in range(NH):
                    qp = psA.tile([128, SQ], F32, tag=f"cq{h % 2}", name="qp")
                    for l in range(L):
                        nc.tensor.matmul(qp[:],
                                         wuq[:, l, h * 128:(h + 1) * 128],
                                         cqT[:, l, :], start=(l == 0),
                                         stop=(l == L - 1))
                    nc.scalar.activation(out=qcT[:, h, :], in_=qp[:],
                                         func=AF.Identity,
                                         bias=buq[:, h:h + 1], scale=1.0)

            # =================== Phase B: key-block loop ===================
            with tc.tile_pool(name="stB", bufs=1) as stB, \
                 tc.tile_pool(name="psB", bufs=1, space="PSUM") as psB:
                wuk = stB.tile([128, L, H], F16)
                nc.sync.dma_start(
                    out=wuk[:], in_=d_wuk.rearrange("(l p) h -> p l h", p=128))
                wuv = stB.tile([128, L, H], F16)
                nc.sync.dma_start(
                    out=wuv[:], in_=d_wuv.rearrange("(l p) h -> p l h", p=128))

                for kb in range(NKB):
                    ksl = slice(kb * KBLK, (kb + 1) * KBLK)
                    # k_cT for this block, per head: [128 d, 512 s] fp16
                    kcT = stB.tile([128, NH, KBLK], F16, tag="kcT",
                                   bufs=2 if reps == 1 else 1)
                    for h in range(NH):
                        kp = psB.tile([128, KBLK], F32, tag="sp", bufs=5,
                                      name="kp")
                        for l in range(L):
                            nc.tensor.matmul(
                                kp[:], wuk[:, l, h * 128:(h + 1) * 128],
                                ckvT[:, l, ksl],
                                start=(l == 0), stop=(l == L - 1))
                        if h % 2 == 0:
                            nc.scalar.activation(out=kcT[:, h, :], in_=kp[:],
                                                 func=AF.Identity,
                                                 bias=buk[:, h:h + 1],
                                                 scale=1.0)
                        else:
                            nc.vector.tensor_scalar_add(kcT[:, h, :], kp[:],
                                                        buk[:, h:h + 1])
                    # v_c natural for this block: [128 s, 4 ss, 2048 dh] fp16
                    vc = stB.tile([128, KSUB, H], F16, tag="vc",
                                  bufs=2 if reps == 1 else 1)
                    for ss in range(KSUB):
                        soff = kb * KBLK + ss * 128
                        for nq in range(4):
                            vp = psB.tile([128, 512], F32, tag="cp", bufs=3,
                                          name="vp")
                            for l in range(L):
                                nc.tensor.matmul(
                                    vp[:], ckvT[:, l, soff:soff + 128],
                                    wuv[:, l, nq * 512:(nq + 1) * 512],
                                    start=(l == 0), stop=(l == L - 1))
                            if nq % 2 == 0:
                                nc.vector.tensor_copy(
                                    vc[:, ss, nq * 512:(nq + 1) * 512], vp[:])
                            else:
                                nc.scalar.activation(
                                    out=vc[:, ss, nq * 512:(nq + 1) * 512],
                                    in_=vp[:], func=AF.Copy, scale=1.0)

                    # attention per head
                    for h in range(NH):
                        exp_ts = []
                        for ks in range(KSUB):
                            kcg = kb * KSUB + ks
                            sp = psB.tile([128, SQ], F32, tag="sp", bufs=5)
                            nc.tensor.matmul(
                                sp[:], kcT[:, h, ks * 128:(ks + 1) * 128],
                                qcT[:, h, :], start=True, stop=False)
                            nc.tensor.matmul(
                                sp[:], krro[:, kcg * 128:(kcg + 1) * 128],
                                qrro[:], start=False, stop=True)
                            et = stB.tile([128, SQ], F16, tag="exp", bufs=8)
                            nc.scalar.activation(out=et[:], in_=sp[:],
                                                 func=AF.Exp,
                                                 bias=maskb[:, kcg:kcg + 1],
                                                 scale=SCALE)
                            exp_ts.append(et)
                        # sums: pairwise fp16 adds, then ones-matmul
                        s01 = stB.tile([128, SQ], F16, tag="s01", bufs=3)
                        s23 = stB.tile([128, SQ], F16, tag="s23", bufs=3)
                        nc.vector.tensor_add(s01[:], exp_ts[0][:], exp_ts[1][:])
                        nc.vector.tensor_add(s23[:], exp_ts[2][:], exp_ts[3][:])
                        nc.vector.tensor_add(s01[:], s01[:], s23[:])
                        op = psB.tile([1, SQ], F32, tag="cp", bufs=3, name="op")
                        nc.tensor.matmul(op[:], ones[:], s01[:],
                                         start=True, stop=True)
                        with nc.allow_low_precision(reason="softmax sums fp16"):
                            if kb == 0:
                                nc.vector.tensor_copy(sums[0:1, h, :], op[:])
                            else:
                                nc.vector.tensor_add(sums[0:1, h, :],
                                                     sums[0:1, h, :], op[:])
                        # ctx
                        cp = psB.tile([128, SQ], F32, tag="cp", bufs=3)
                        for ks in range(KSUB):
                            nc.tensor.matmul(
                                cp[:], vc[:, ks, h * 128:(h + 1) * 128],
                                exp_ts[ks][:],
                                start=(ks == 0), stop=(ks == KSUB - 1))
                        if kb == 0:
                            nc.vector.tensor_copy(ctxa[:, h, :], cp[:])
                        else:
                            nc.vector.tensor_add(ctxa[:, h, :],
                                                 ctxa[:, h, :], cp[:])

            # =================== Phase C: normalize + W_O ===================
            with tc.tile_pool(name="stC", bufs=1) as stC, \
                 tc.tile_pool(name="psC", bufs=1, space="PSUM") as psC, \
                 tc.tile_pool(name="drC", bufs=1, space="DRAM") as drC:
                rdram = drC.tile([NH, SQ], F16)
                for h in range(NH):
                    with nc.allow_low_precision(reason="softmax recip fp16"):
                        nc.vector.reciprocal(sums[0:1, h, :], sums[0:1, h, :])
                    nc.gpsimd.dma_start(out=rdram[h:h + 1, :],
                                        in_=sums[0:1, h, :])
                    rb = stC.tile([128, SQ], F16, tag="rb", bufs=4)
                    row = rdram[h:h + 1, :]
                    bcast = bass.AP(tensor=row.tensor, offset=row.offset,
                                    ap=[[0, 128]] + [list(p) for p in row.ap[1:]])
                    nc.gpsimd.dma_start(out=rb[:], in_=bcast)
                    nc.vector.tensor_mul(ctxa[:, h, :], ctxa[:, h, :], rb[:])
                    nc.scalar.activation(out=ctxf[:, h, :], in_=ctxa[:, h, :],
                                         func=AF.Identity,
                                         bias=buv[:, h:h + 1], scale=1.0)
                for nh in range(2):
                    ops = [[psC.tile([128, 512], F32, tag=f"o{q4}{n2}",
                                     name=f"o{q4}{n2}")
                            for n2 in range(2)] for q4 in range(4)]
                    for q4 in range(4):
                        for n2 in range(2):
                            nc.tensor.matmul(
                                ops[q4][n2][:], onesr[:],
                                bo[0:1, nh * 1024 + n2 * 512:
                                   nh * 1024 + (n2 + 1) * 512],
                                start=True, stop=False)
                    for dh in range(NH):
                        wo = stC.tile([128, 1024], F16, tag="wo", bufs=6)
                        nc.sync.dma_start(
                            out=wo[:],
                            in_=d_wo[dh * 128:(dh + 1) * 128,
                                     nh * 1024:(nh + 1) * 1024])
                        for q4 in range(4):
                            for n2 in range(2):
                                nc.tensor.matmul(
                                    ops[q4][n2][:],
                                    ctxf[:, dh, q4 * 128:(q4 + 1) * 128],
                                    wo[:, n2 * 512:(n2 + 1) * 512],
                                    start=False, stop=(dh == NH - 1))
                    for q4 in range(4):
                        ot = stC.tile([128, 1024], F32, tag="ot", bufs=4)
                        for n2 in range(2):
                            nc.scalar.activation(
                                out=ot[:, n2 * 512:(n2 + 1) * 512],
                                in_=ops[q4][n2][:], func=AF.Copy, scale=1.0)
                        nc.sync.dma_start(
                            out=d_out[q4 * 128:(q4 + 1) * 128,
                                      nh * 1024:(nh + 1) * 1024],
                            in_=ot[:])
    _split_multi_waits(nc)
    return nc


_cache = {}


def _get_nc():
    if "nc" not in _cache:
        _cache["nc"] = build()
    return _cache["nc"]


def _host_prep(hidden_states, attention_mask, W_DKV, b_DKV, W_DQ, b_DQ,
               W_UK, b_UK, W_UV, b_UV, W_UQ, b_UQ,
               W_KR, b_KR, W_QR, b_QR, W_O, b_O):
    f32 = np.float32
    f16 = np.float16
    hidden = np.asarray(hidden_states, f32)
    mask = np.asarray(attention_mask)

    inv = 1.0 / (10000.0 ** (np.arange(0, D, 2, dtype=np.float64) / D))
    ang = inv[:, None] * np.arange(S, dtype=np.float64)[None, :]   # [64, S]
    cos = np.cos(ang)
    sin = np.sin(ang)
    cos2 = np.concatenate([cos, cos], 0).astype(f16)      # [128, S]
    sins = np.concatenate([-sin, sin], 0).astype(f16)

    shared = {
        "wdkv": np.ascontiguousarray(np.asarray(W_DKV, f32).astype(f16)),
        "wdq": np.ascontiguousarray(np.asarray(W_DQ, f32).astype(f16)),
        "wkr": np.ascontiguousarray(np.asarray(W_KR, f32).astype(f16)),
        "wqr": np.ascontiguousarray(np.asarray(W_QR, f32).astype(f16)),
        "wuk": np.ascontiguousarray(np.asarray(W_UK, f32).astype(f16)),
        "wuv": np.ascontiguousarray(np.asarray(W_UV, f32).astype(f16)),
        "wuq": np.ascontiguousarray(np.asarray(W_UQ, f32).astype(f16)),
        "wo": np.ascontiguousarray(np.asarray(W_O, f32).astype(f16)),
        "cos2": np.ascontiguousarray(cos2),
        "sins": np.ascontiguousarray(sins),
        "bdkv": np.ascontiguousarray(np.asarray(b_DKV, f32).reshape(L, 128).T),
        "bdq": np.ascontiguousarray(np.asarray(b_DQ, f32).reshape(L, 128).T),
        "buk": np.ascontiguousarray(np.asarray(b_UK, f32).reshape(NH, 128).T),
        "buq": np.ascontiguousarray(np.asarray(b_UQ, f32).reshape(NH, 128).T),
        "buv": np.ascontiguousarray(np.asarray(b_UV, f32).reshape(NH, 128).T),
        "bkr": np.asarray(b_KR, f32).reshape(128, 1),
        "bqr": np.asarray(b_QR, f32).reshape(128, 1),
        "bo": np.asarray(b_O, f32).reshape(1, H),
        "ones128": np.ones((128, 1), f16),
        "onesrow": np.ones((1, 128), f32),
    }
    per_batch = {}
    for b in range(B):
        hT = np.ascontiguousarray(hidden[b].T.astype(f16))
        mb = np.where(np.asarray(mask[b]) == 0, -1e30, 0.0).astype(f32) - SHIFT
        per_batch[b] = {
            "hT": hT,
            "maskb": np.ascontiguousarray(mb.reshape(E, 128).T),
        }
    in_maps = []
    for core in range(8):
        b, qc = core // 4, core % 4
        qsl = slice(qc * SQ, (qc + 1) * SQ)
        m = dict(shared)
        m.update(per_batch[b])
        m["hTq"] = np.ascontiguousarray(per_batch[b]["hT"][:, qsl])
        m["cos2q"] = np.ascontiguousarray(cos2[:, qsl])
        m["sinsq"] = np.ascontiguousarray(sins[:, qsl])
        in_maps.append(m)
    return in_maps


def kernel(**inputs):
    nc = _get_nc()
    in_maps = _host_prep(**inputs)
    res = run_bass_kernel_spmd(nc, in_maps, list(range(8)))
    out = np.empty((B, S, H), np.float32)
    for core in range(8):
        b, qc = core // 4, core % 4
        out[b, qc * SQ:(qc + 1) * SQ, :] = res.results[core]["out"]
    return out



# revision 6
# speedup vs baseline: 1694.3765x; 1694.3765x over previous
"""MultiHeadLatentAttention on 8 trn2 NeuronCores (Bass/Tile).

Sharding: core = (b, qc) with b = core//4 (data parallel over batch),
qc = core%4 (query-chunk of 512 positions). Each core computes the
latent c_kv / k_c / v_c / rope-k ONLY for its own 512-key block, then a
grouped AllGather ([[0..3],[4..7]]) exchanges {k_cT, v_c, k_r_rope}
within the batch group, so the 4x replicated up-projection work of the
all-local design is gone. The rope score contribution q_r.k_r is
head-independent; it is applied as exp(s) = exp(s_c)*exp(s_r) with one
expR tile per key chunk instead of a second matmul per head.

Device layouts (partition dim first):
  hTq     [H, SQ]   hidden[b][:, own 512 cols]^T fp16 (host prep)
  ckvT    [LAT, SQ] latent KV for own block, fp16
  kcT_t   [D, NH, KBLK] per key-block (gathered) fp16
  vc_t    [KBLK(p), KSUB, H] natural fp16 (gathered)
  sp      [Sk, SQ]  scores, keys on partitions; softmax normalized late
  ctxa    [D, NH, SQ] f32 accumulator -> fp16 normalized -> W_O
Projections contract over H/LAT in PSUM fp32. exp() runs on ACT; the key
mask + constant logit shift ride on the rope-side exp bias.
"""
import math
import numpy as np
from contextlib import ExitStack

import concourse.bass as bass
import concourse.mybir as mybir
import concourse.tile as tile
from concourse.bass_utils import run_bass_kernel_spmd

F32R = mybir.dt.float32r
F32 = mybir.dt.float32
F16 = mybir.dt.float16
AF = mybir.ActivationFunctionType

H, NH, LAT = 2048, 16, 512
D = H // NH            # 128
B, S = 2, 2048
SQ = S // 4            # 512 queries per core
E = H // 128           # 16 contraction chunks over H
L = LAT // 128         # 4 chunks over LAT
KBLK = 512             # key block (= own shard)
NKB = S // KBLK        # 4
KSUB = KBLK // 128     # 4 key sub-chunks of 128
SCALE = 1.0 / math.sqrt(D)
SHIFT = 8.0            # softmax logit shift (cancels in normalization)
GROUPS = [[0, 1, 2, 3], [4, 5, 6, 7]]

_n_split = 0


def _split_multi_waits(nc):
    """walrus in this container allows ONE sync wait per instruction; Tile
    attaches several. Hoist extras onto single-wait NoOps just before."""
    global _n_split
    total = 0
    for f in nc.m.functions:
        for bb in f.blocks:
            out = []
            changed = False
            for inst in bb.instructions:
                si = inst.sync_info
                if si is not None and len(si.on_wait) > 1:
                    changed = True
                    waits = list(si.on_wait)
                    for w in waits[:-1]:
                        _n_split += 1
                        total += 1
                        nop = mybir.InstNoOp(
                            name=f"wsplit-{_n_split}", text_hint="waitsplit")
                        nop.engine = inst.engine
                        nop.sync_info = mybir.SyncInfo(on_wait=[w], on_update=[])
                        nc.register_instruction(nop)
                        out.append(nop)
                    inst.sync_info = mybir.SyncInfo(
                        on_wait=[waits[-1]], on_update=list(si.on_update))
                out.append(inst)
            if changed:
                bb.instructions = out
    return total


def build(reps=1):
    nc = bass.Bass()

    d_hTq = nc.dram_tensor("hTq", [H, SQ], F16, kind="ExternalInput")
    d_wdkv = nc.dram_tensor("wdkv", [H, LAT], F16, kind="ExternalInput")
    d_wdq = nc.dram_tensor("wdq", [H, LAT], F16, kind="ExternalInput")
    d_wkr = nc.dram_tensor("wkr", [H, D], F16, kind="ExternalInput")
    d_wqr = nc.dram_tensor("wqr", [H, D], F16, kind="ExternalInput")
    d_wuk = nc.dram_tensor("wuk", [LAT, H], F16, kind="ExternalInput")
    d_wuv = nc.dram_tensor("wuv", [LAT, H], F16, kind="ExternalInput")
    d_wuq = nc.dram_tensor("wuq", [LAT, H], F16, kind="ExternalInput")
    d_wo = nc.dram_tensor("wo", [H, H], F16, kind="ExternalInput")
    d_cos2q = nc.dram_tensor("cos2q", [D, SQ], F16, kind="ExternalInput")
    d_sinsq = nc.dram_tensor("sinsq", [D, SQ], F16, kind="ExternalInput")
    d_maskb = nc.dram_tensor("maskb", [128, E], F32, kind="ExternalInput")
    d_bdkv = nc.dram_tensor("bdkv", [128, L], F32, kind="ExternalInput")
    d_bdq = nc.dram_tensor("bdq", [128, L], F32, kind="ExternalInput")
    d_buk = nc.dram_tensor("buk", [128, NH], F32, kind="ExternalInput")
    d_buq = nc.dram_tensor("buq", [128, NH], F32, kind="ExternalInput")
    d_buv = nc.dram_tensor("buv", [128, NH], F32, kind="ExternalInput")
    d_bkr = nc.dram_tensor("bkr", [128, 1], F32, kind="ExternalInput")
    d_bqr = nc.dram_tensor("bqr", [128, 1], F32, kind="ExternalInput")
    d_bo = nc.dram_tensor("bo", [1, H], F32R, kind="ExternalInput")
    d_ones = nc.dram_tensor("ones128", [128, 1], F16, kind="ExternalInput")
    d_onesr = nc.dram_tensor("onesrow", [1, 128], F32R, kind="ExternalInput")
    d_out = nc.dram_tensor("out", [SQ, H], F32, kind="ExternalOutput")

    with tile.TileContext(nc) as tc, ExitStack() as es:
        perm = es.enter_context(tc.tile_pool(name="perm", bufs=1))
        drB = es.enter_context(tc.tile_pool(name="drB", bufs=1, space="DRAM"))

        qcT = perm.tile([128, NH, SQ], F16)        # 16 KB/p
        ctxa = perm.tile([128, NH, SQ], F32)       # 32
        ctxf = perm.tile([128, NH, SQ], F16)       # 16
        sums = perm.tile([1, NH, SQ], F16)
        qrro = perm.tile([128, SQ], F16)           # 1
        ckvT = perm.tile([128, L, SQ], F16)        # 4
        krro = perm.tile([128, SQ], F16)           # 1

        maskb = perm.tile([128, E], F32)
        bdkv = perm.tile([128, L], F32)
        bdq = perm.tile([128, L], F32)
        buk = perm.tile([128, NH], F32)
        buq = perm.tile([128, NH], F32)
        buv = perm.tile([128, NH], F32)
        bkr = perm.tile([128, 1], F32)
        bqr = perm.tile([128, 1], F32)
        bo = perm.tile([1, H], F32R)
        ones = perm.tile([128, 1], F16)
        onesr = perm.tile([1, 128], F32R)
        consts = [(maskb, d_maskb), (bdkv, d_bdkv), (bdq, d_bdq),
                  (buk, d_buk), (buq, d_buq), (buv, d_buv),
                  (bkr, d_bkr), (bqr, d_bqr), (bo, d_bo),
                  (ones, d_ones), (onesr, d_onesr)]

        # bounce buffers for the grouped AllGather: [kcT | vc | krro]
        inb = drB.tile([128, 33, 512], F16)        # own contribution
        outb = drB.tile([512, 33, 512], F16)       # gathered (block r at rows r)

        for _rep in range(reps):
            # ============ Phase A: local projections (own 512 cols) ========
            with tc.tile_pool(name="stA", bufs=1) as stA, \
                 tc.tile_pool(name="psA", bufs=1, space="PSUM") as psA:
                ht = stA.tile([128, E, SQ], F16)
                nc.sync.dma_start(
                    out=ht[:], in_=d_hTq.rearrange("(e p) q -> p e q", p=128))
                wdkv = stA.tile([128, E, LAT], F16)
                nc.sync.dma_start(
                    out=wdkv[:],
                    in_=d_wdkv.rearrange("(e p) l -> p e l", p=128))
                wkr = stA.tile([128, E, D], F16)
                nc.sync.dma_start(
                    out=wkr[:], in_=d_wkr.rearrange("(e p) d -> p e d", p=128))
                if _rep == 0:
                    for t, d in consts:
                        nc.gpsimd.dma_start(out=t[:], in_=d[:])
                wdq = stA.tile([128, E, LAT], F16)
                nc.gpsimd.dma_start(
                    out=wdq[:], in_=d_wdq.rearrange("(e p) l -> p e l", p=128))
                wqr = stA.tile([128, E, D], F16)
                nc.gpsimd.dma_start(
                    out=wqr[:], in_=d_wqr.rearrange("(e p) d -> p e d", p=128))
                wuq = stA.tile([128, L, H], F16)
                nc.gpsimd.dma_start(
                    out=wuq[:], in_=d_wuq.rearrange("(l p) h -> p l h", p=128))

                krraw = stA.tile([128, SQ], F32, tag="krraw")
                cqT = stA.tile([128, L, SQ], F16)
                qrraw = stA.tile([128, SQ], F32, tag="qrraw")

                # --- A1: c_kvT + raw k_rT (own block) ---
                ps = [psA.tile([128, SQ], F32, tag=f"ckv{l}", name=f"ckv{l}")
                      for l in range(L)]
                kps = psA.tile([128, SQ], F32, tag="krp", name="krp")
                for e in range(E):
                    for l in range(L):
                        nc.tensor.matmul(
                            ps[l][:], wdkv[:, e, l * 128:(l + 1) * 128],
                            ht[:, e, :], start=(e == 0), stop=(e == E - 1))
                    nc.tensor.matmul(kps[:], wkr[:, e, :], ht[:, e, :],
                                     start=(e == 0), stop=(e == E - 1))
                for l in range(L):
                    nc.scalar.activation(
                        out=ckvT[:, l, :], in_=ps[l][:], func=AF.Identity,
                        bias=bdkv[:, l:l + 1], scale=1.0)
                nc.scalar.activation(
                    out=krraw[:], in_=kps[:],
                    func=AF.Identity, bias=bkr[:], scale=1.0)

                # --- A2: c_qT + raw q_rT (same psum tags, reused) ---
                qs = [psA.tile([128, SQ], F32, tag=f"ckv{l}", name=f"cq{l}")
                      for l in range(L)]
                qps = psA.tile([128, SQ], F32, tag="krp", name="qrp")
                for e in range(E):
                    for l in range(L):
                        nc.tensor.matmul(
                            qs[l][:], wdq[:, e, l * 128:(l + 1) * 128],
                            ht[:, e, :], start=(e == 0), stop=(e == E - 1))
                    nc.tensor.matmul(qps[:], wqr[:, e, :], ht[:, e, :],
                                     start=(e == 0), stop=(e == E - 1))
                for l in range(L):
                    nc.scalar.activation(
                        out=cqT[:, l, :], in_=qs[l][:], func=AF.Identity,
                        bias=bdq[:, l:l + 1], scale=1.0)
                nc.scalar.activation(out=qrraw[:], in_=qps[:],
                                     func=AF.Identity, bias=bqr[:], scale=1.0)

                # --- A3: rope for k and q: out = raw*cos2 + swap(raw)*sins ---
                cos2q = stA.tile([128, SQ], F16, tag="cos2q")
                sinsq = stA.tile([128, SQ], F16, tag="sinsq")
                nc.sync.dma_start(out=cos2q[:], in_=d_cos2q[:])
                nc.sync.dma_start(out=sinsq[:], in_=d_sinsq[:])
                krsw = stA.tile([128, SQ], F32, tag="krsw")
                nc.gpsimd.dma_start(out=krsw[0:64, :], in_=krraw[64:128, :])
                nc.gpsimd.dma_start(out=krsw[64:128, :], in_=krraw[0:64, :])
                nc.vector.tensor_mul(krraw[:], krraw[:], cos2q[:])
                nc.vector.tensor_mul(krsw[:], krsw[:], sinsq[:])
                nc.vector.tensor_add(krro[:], krraw[:], krsw[:])
                qrsw = stA.tile([128, SQ], F32, tag="qrsw")
                nc.gpsimd.dma_start(out=qrsw[0:64, :], in_=qrraw[64:128, :])
                nc.gpsimd.dma_start(out=qrsw[64:128, :], in_=qrraw[0:64, :])
                nc.vector.tensor_mul(qrraw[:], qrraw[:], cos2q[:])
                nc.vector.tensor_mul(qrsw[:], qrsw[:], sinsq[:])
                nc.vector.tensor_add(qrro[:], qrraw[:], qrsw[:])

                # --- A4: q_cT (fp16) = W_UQ^T @ c_qT ---
                for h in range(NH):
                    qp = psA.tile([128, SQ], F32, tag=f"qp{h % 2}", name="qp")
                    for l in range(L):
                        nc.tensor.matmul(qp[:],
                                         wuq[:, l, h * 128:(h + 1) * 128],
                                         cqT[:, l, :], start=(l == 0),
                                         stop=(l == L - 1))
                    nc.scalar.activation(out=qcT[:, h, :], in_=qp[:],
                                         func=AF.Identity,
                                         bias=buq[:, h:h + 1], scale=1.0)

            # ============ Phase B1: own-block k_c / v_c + gather ===========
            with tc.tile_pool(name="stB1", bufs=1) as stB1, \
                 tc.tile_pool(name="psB1", bufs=1, space="PSUM") as psB1:
                wuk = stB1.tile([128, L, H], F16)
                nc.sync.dma_start(
                    out=wuk[:],
                    in_=d_wuk.rearrange("(l p) h -> p l h", p=128))
                wuv = stB1.tile([128, L, H], F16)
                nc.sync.dma_start(
                    out=wuv[:],
                    in_=d_wuv.rearrange("(l p) h -> p l h", p=128))

                kco = stB1.tile([128, NH, KBLK], F16)
                for h in range(NH):
                    kp = psB1.tile([128, KBLK], F32, tag="kp", bufs=3,
                                   name="kp")
                    for l in range(L):
                        nc.tensor.matmul(
                            kp[:], wuk[:, l, h * 128:(h + 1) * 128],
                            ckvT[:, l, :],
                            start=(l == 0), stop=(l == L - 1))
                    if h % 2 == 0:
                        nc.scalar.activation(out=kco[:, h, :], in_=kp[:],
                                             func=AF.Identity,
                                             bias=buk[:, h:h + 1],
                                             scale=1.0)
                    else:
                        nc.vector.tensor_scalar_add(kco[:, h, :], kp[:],
                                                    buk[:, h:h + 1])
                vco = stB1.tile([128, KSUB, H], F16)
                for ss in range(KSUB):
                    for nq in range(4):
                        vp = psB1.tile([128, 512], F32, tag="vp", bufs=3,
                                       name="vp")
                        for l in range(L):
                            nc.tensor.matmul(
                                vp[:], ckvT[:, l, ss * 128:(ss + 1) * 128],
                                wuv[:, l, nq * 512:(nq + 1) * 512],
                                start=(l == 0), stop=(l == L - 1))
                        if nq % 2 == 0:
                            nc.vector.tensor_copy(
                                vco[:, ss, nq * 512:(nq + 1) * 512], vp[:])
                        else:
                            nc.scalar.activation(
                                out=vco[:, ss, nq * 512:(nq + 1) * 512],
                                in_=vp[:], func=AF.Copy, scale=1.0)

                # bounce out + grouped AllGather (blocks land in order)
                nc.gpsimd.dma_start(out=inb[:, 0:16, :], in_=kco[:])
                nc.gpsimd.dma_start(out=inb[:, 16:32, :], in_=vco[:])
                nc.gpsimd.dma_start(out=inb[:, 32, :], in_=krro[:])
                nc.gpsimd.collective_compute(
                    "AllGather", mybir.AluOpType.bypass,
                    replica_groups=GROUPS,
                    ins=[inb[:]], outs=[outb[:]])

            # =================== Phase B2: attention over blocks ===========
            with tc.tile_pool(name="stB", bufs=1) as stB, \
                 tc.tile_pool(name="psB", bufs=1, space="PSUM") as psB:
                for kb in range(NKB):
                    rsl = slice(kb * 128, (kb + 1) * 128)
                    kcT = stB.tile([128, NH, KBLK], F16, tag="kcT", bufs=2)
                    eng = nc.sync if kb % 2 == 0 else nc.scalar
                    eng.dma_start(out=kcT[:], in_=outb[rsl, 0:16, :])
                    vc = stB.tile([128, KSUB, H], F16, tag="vc", bufs=2)
                    eng.dma_start(out=vc[:], in_=outb[rsl, 16:32, :])
                    krt = stB.tile([128, KBLK], F16, tag="krt", bufs=2)
                    eng.dma_start(out=krt[:], in_=outb[rsl, 32, :])

                    # head-independent rope factor exp(scale*s_r + mask-shift)
                    expR = stB.tile([128, KSUB, SQ], F16, tag="expR", bufs=2)
                    for ks in range(KSUB):
                        kcg = kb * KSUB + ks
                        rp = psB.tile([128, SQ], F32, tag="sp", bufs=4,
                                      name="rp")
                        nc.tensor.matmul(
                            rp[:], krt[:, ks * 128:(ks + 1) * 128],
                            qrro[:], start=True, stop=True)
                        nc.scalar.activation(out=expR[:, ks, :], in_=rp[:],
                                             func=AF.Exp,
                                             bias=maskb[:, kcg:kcg + 1],
                                             scale=SCALE)

                    for h in range(NH):
                        exp_ts = []
                        for ks in range(KSUB):
                            sp = psB.tile([128, SQ], F32, tag="sp", bufs=4)
                            nc.tensor.matmul(
                                sp[:], kcT[:, h, ks * 128:(ks + 1) * 128],
                                qcT[:, h, :], start=True, stop=True)
                            er = stB.tile([128, SQ], F16, tag="expraw", bufs=4)
                            nc.scalar.activation(out=er[:], in_=sp[:],
                                                 func=AF.Exp, scale=SCALE)
                            et = stB.tile([128, SQ], F16, tag="exp", bufs=8)
                            nc.vector.tensor_mul(et[:], er[:],
                                                 expR[:, ks, :])
                            exp_ts.append(et)
                        # sums: pairwise fp16 adds, then ones-matmul
                        s01 = stB.tile([128, SQ], F16, tag="s01", bufs=3)
                        s23 = stB.tile([128, SQ], F16, tag="s23", bufs=3)
                        nc.vector.tensor_add(s01[:], exp_ts[0][:], exp_ts[1][:])
                        nc.vector.tensor_add(s23[:], exp_ts[2][:], exp_ts[3][:])
                        nc.vector.tensor_add(s01[:], s01[:], s23[:])
                        op = psB.tile([1, SQ], F32, tag="cp", bufs=3, name="op")
                        nc.tensor.matmul(op[:], ones[:], s01[:],
                                         start=True, stop=True)
                        with nc.allow_low_precision(reason="softmax sums fp16"):
                            if kb == 0:
                                nc.vector.tensor_copy(sums[0:1, h, :], op[:])
                            else:
                                nc.vector.tensor_add(sums[0:1, h, :],
                                                     sums[0:1, h, :], op[:])
                        # ctx
                        cp = psB.tile([128, SQ], F32, tag="cp", bufs=3)
                        for ks in range(KSUB):
                            nc.tensor.matmul(
                                cp[:], vc[:, ks, h * 128:(h + 1) * 128],
                                exp_ts[ks][:],
                                start=(ks == 0), stop=(ks == KSUB - 1))
                        if kb == 0:
                            nc.vector.tensor_copy(ctxa[:, h, :], cp[:])
                        else:
                            nc.vector.tensor_add(ctxa[:, h, :],
                                                 ctxa[:, h, :], cp[:])

            # =================== Phase C: normalize + W_O ===================
            with tc.tile_pool(name="stC", bufs=1) as stC, \
                 tc.tile_pool(name="psC", bufs=1, space="PSUM") as psC, \
                 tc.tile_pool(name="drC", bufs=1, space="DRAM") as drC:
                rdram = drC.tile([NH, SQ], F16)
                for h in range(NH):
                    with nc.allow_low_precision(reason="softmax recip fp16"):
                        nc.vector.reciprocal(sums[0:1, h, :], sums[0:1, h, :])
                    nc.gpsimd.dma_start(out=rdram[h:h + 1, :],
                                        in_=sums[0:1, h, :])
                    rb = stC.tile([128, SQ], F16, tag="rb", bufs=4)
                    row = rdram[h:h + 1, :]
                    bcast = bass.AP(tensor=row.tensor, offset=row.offset,
                                    ap=[[0, 128]] + [list(p) for p in row.ap[1:]])
                    nc.gpsimd.dma_start(out=rb[:], in_=bcast)
                    nc.vector.tensor_mul(ctxa[:, h, :], ctxa[:, h, :], rb[:])
                    nc.scalar.activation(out=ctxf[:, h, :], in_=ctxa[:, h, :],
                                         func=AF.Identity,
                                         bias=buv[:, h:h + 1], scale=1.0)
                for nh in range(2):
                    ops = [[psC.tile([128, 512], F32, tag=f"o{q4}{n2}",
                                     name=f"o{q4}{n2}")
                            for n2 in range(2)] for q4 in range(4)]
                    for q4 in range(4):
                        for n2 in range(2):
                            nc.tensor.matmul(
                                ops[q4][n2][:], onesr[:],
                                bo[0:1, nh * 1024 + n2 * 512:
                                   nh * 1024 + (n2 + 1) * 512],
                                start=True, stop=False)
                    for dh in range(NH):
                        wo = stC.tile([128, 1024], F16, tag="wo", bufs=6)
                        nc.sync.dma_start(
                            out=wo[:],
                            in_=d_wo[dh * 128:(dh + 1) * 128,
                                     nh * 1024:(nh + 1) * 1024])
                        for q4 in range(4):
                            for n2 in range(2):
                                nc.tensor.matmul(
                                    ops[q4][n2][:],
                                    ctxf[:, dh, q4 * 128:(q4 + 1) * 128],
                                    wo[:, n2 * 512:(n2 + 1) * 512],
                                    start=False, stop=(dh == NH - 1))
                    for q4 in range(4):
                        ot = stC.tile([128, 1024], F32, tag="ot", bufs=4)
                        for n2 in range(2):
                            nc.scalar.activation(
                                out=ot[:, n2 * 512:(n2 + 1) * 512],
                                in_=ops[q4][n2][:], func=AF.Copy, scale=1.0)
                        nc.sync.dma_start(
                            out=d_out[q4 * 128:(q4 + 1) * 128,
                                      nh * 1024:(nh + 1) * 1024],
                            in_=ot[:])
    _split_multi_waits(nc)
    return nc


_cache = {}


def _get_nc():
    if "nc" not in _cache:
        _cache["nc"] = build()
    return _cache["nc"]


def _host_prep(hidden_states, attention_mask, W_DKV, b_DKV, W_DQ, b_DQ,
               W_UK, b_UK, W_UV, b_UV, W_UQ, b_UQ,
               W_KR, b_KR, W_QR, b_QR, W_O, b_O):
    f32 = np.float32
    f16 = np.float16
    hidden = np.asarray(hidden_states, f32)
    mask = np.asarray(attention_mask)

    inv = 1.0 / (10000.0 ** (np.arange(0, D, 2, dtype=np.float64) / D))
    ang = inv[:, None] * np.arange(S, dtype=np.float64)[None, :]   # [64, S]
    cos = np.cos(ang)
    sin = np.sin(ang)
    cos2 = np.concatenate([cos, cos], 0).astype(f16)      # [128, S]
    sins = np.concatenate([-sin, sin], 0).astype(f16)

    shared = {
        "wdkv": np.ascontiguousarray(np.asarray(W_DKV, f32).astype(f16)),
        "wdq": np.ascontiguousarray(np.asarray(W_DQ, f32).astype(f16)),
        "wkr": np.ascontiguousarray(np.asarray(W_KR, f32).astype(f16)),
        "wqr": np.ascontiguousarray(np.asarray(W_QR, f32).astype(f16)),
        "wuk": np.ascontiguousarray(np.asarray(W_UK, f32).astype(f16)),
        "wuv": np.ascontiguousarray(np.asarray(W_UV, f32).astype(f16)),
        "wuq": np.ascontiguousarray(np.asarray(W_UQ, f32).astype(f16)),
        "wo": np.ascontiguousarray(np.asarray(W_O, f32).astype(f16)),
        "bdkv": np.ascontiguousarray(np.asarray(b_DKV, f32).reshape(L, 128).T),
        "bdq": np.ascontiguousarray(np.asarray(b_DQ, f32).reshape(L, 128).T),
        "buk": np.ascontiguousarray(np.asarray(b_UK, f32).reshape(NH, 128).T),
        "buq": np.ascontiguousarray(np.asarray(b_UQ, f32).reshape(NH, 128).T),
        "buv": np.ascontiguousarray(np.asarray(b_UV, f32).reshape(NH, 128).T),
        "bkr": np.asarray(b_KR, f32).reshape(128, 1),
        "bqr": np.asarray(b_QR, f32).reshape(128, 1),
        "bo": np.asarray(b_O, f32).reshape(1, H),
        "ones128": np.ones((128, 1), f16),
        "onesrow": np.ones((1, 128), f32),
    }
    per_batch = {}
    for b in range(B):
        hT = np.ascontiguousarray(hidden[b].T.astype(f16))
        mb = np.where(np.asarray(mask[b]) == 0, -1e30, 0.0).astype(f32) - SHIFT
        per_batch[b] = {
            "hT": hT,
            "maskb": np.ascontiguousarray(mb.reshape(E, 128).T),
        }
    in_maps = []
    for core in range(8):
        b, qc = core // 4, core % 4
        qsl = slice(qc * SQ, (qc + 1) * SQ)
        m = dict(shared)
        m["maskb"] = per_batch[b]["maskb"]
        m["hTq"] = np.ascontiguousarray(per_batch[b]["hT"][:, qsl])
        m["cos2q"] = np.ascontiguousarray(cos2[:, qsl])
        m["sinsq"] = np.ascontiguousarray(sins[:, qsl])
        in_maps.append(m)
    return in_maps


def kernel(**inputs):
    nc = _get_nc()
    in_maps = _host_prep(**inputs)
    res = run_bass_kernel_spmd(nc, in_maps, list(range(8)))
    out = np.empty((B, S, H), np.float32)
    for core in range(8):
        b, qc = core // 4, core % 4
        out[b, qc * SQ:(qc + 1) * SQ, :] = res.results[core]["out"]
    return out
